# revision 1
# baseline (speedup 1.0000x reference)
"""Trainium2 Bass kernel for nn_AdaFeatBlock (modulated deformable-conv block).

Sharding: data-parallel over batch — 8 samples -> 8 NeuronCores, all weights
replicated; each core computes its sample end-to-end, host stacks outputs.

Per-core pipeline (one sample, x [64,128,128]):
  1. x -> bf16 "half-split" padded layout: partition h*64+c = channel c of
     image-half h; free = 76 stored rows (h*64-6 .. h*64+69) x 130 cols
     (-1..128), zero-padded borders.
  2. offset/mask 3x3 conv = 9 shifted matmuls, block-diagonal [128, 54]
     lhsT (both halves at once), PSUM-accumulated. Output row order per
     half: [off_y k0..8 | off_x k0..8 | mask k0..8].
  3. Coordinate math on [18, 8192] tiles (partition = (half, k)): bilinear
     corner weights (mask-modulated, zero outside the image via the
     zero-padded gather table) and 2x2-quad table indices.
  4. Quad gather table Q[128, 10032, 4] bf16: 2x2 pixel blocks at all 4
     row/col parities. ap_gather (d=2 f32 view = 8B quad) fetches a
     point's 4 corners for 16 channels/core; 8 Q7 cores cover
     128 partitions = 2 halves x 64 channels.
  5. Per (k, 512-px block): corner-weight rows broadcast to 128 partitions
     via a tiny selector matmul into PSUM; fused PSUM-read multiply into
     gathered corners; 3 adds -> modulated bilinear sample "val".
  6. Deformable einsum: per k a [128->128, 512] matmul with block-diagonal
     channel-duplicated w_dc, PSUM-accumulated over all 9 k. + b_dc -> out.
"""

import numpy as np

import concourse.bass as bass
import concourse.tile as tile
from concourse import mybir
from concourse.bass_utils import run_bass_kernel_spmd
from concourse import library_config
from concourse.library_overlay import lower_extended_insts
from concourse.vector_clock import ScopedClock

AF = mybir.ActivationFunctionType
ALU = mybir.AluOpType
DT = mybir.dt

B, C, H, W = 8, 64, 128, 128
O = 64
K = 3
KF = 9
NCORES = 8
HALF = H // 2
NPIX = H * W // 2              # 8192 pixels per half
ROWS_ST = 76                   # stored rows per half
PITCH = 130                    # stored cols (-1..128)
RY_N, RX_N = 38, 66
NBLK = 4 * RY_N * RX_N         # 10032
GCHUNK = 2048
SUB = 512
S16 = NPIX // 16               # idx ints per partition per k


def _install_compat():
    """This walrus build accepts at most ONE sync-wait per instruction."""
    if getattr(tile.TileContext, "_adafeat_patched", False):
        return
    _orig_lower = tile.TileContext._lower_ordered_insts

    def _split_waits(nc, ordered):
        for insts in ordered.values():
            new_insts = []
            for inst in insts:
                si = inst.sync_info
                if si is not None and si.on_wait and len(si.on_wait) > 1:
                    waits = list(si.on_wait)
                    for w in waits[:-1]:
                        nop = mybir.InstNoOp(name=f"I-{nc.next_id()}", ins=[], outs=[])
                        nop.engine = inst.engine
                        nop.sync_info = mybir.SyncInfo(on_wait=[w], on_update=[])
                        new_insts.append(nop)
                    inst.sync_info = mybir.SyncInfo(
                        on_wait=[waits[-1]], on_update=list(si.on_update)
                    )
                new_insts.append(inst)
            insts[:] = new_insts

    def _lower_split(self, ordered):
        _split_waits(self.nc, ordered)
        return _orig_lower(self, ordered)

    def _drain_split(self, tick_clock, wait_clock):
        carrier = self.nc.sync.nop(nofuse=True)
        wait_clock.add_sem_waits(
            carrier.ins, ScopedClock({None: tick_clock.global_clock})
        )
        si = carrier.ins.sync_info
        if si is not None and si.on_wait and len(si.on_wait) > 1:
            waits = list(si.on_wait)
            carrier.ins.sync_info = mybir.SyncInfo(
                on_wait=waits[:1], on_update=list(si.on_update)
            )
            for w in waits[1:]:
                extra = self.nc.sync.nop(nofuse=True)
                extra.ins.sync_info = mybir.SyncInfo(on_wait=[w], on_update=[])
        self.nc.sync.drain()
        self.nc.all_engine_barrier()
        popped = self.nc._tile_sem_poison_stack.pop()
        assert popped is self._sem_poison
        self.nc.clear_and_free_semaphores(list(self.sems.allocated().values()))
        self.nc.all_engine_barrier()

    tile.TileContext._lower_ordered_insts = _lower_split
    tile.TileContext._drain_and_barrier = _drain_split
    tile.TileContext._adafeat_patched = True


def _emit(nc, tc, x_ext, wom_ext, bom_ext, wdc_ext, bdc_ext, out_ext):
    _iotas = []

    with tc.tile_pool(name="persist", bufs=1) as persist:
        x_sb = persist.tile([128, ROWS_ST * PITCH], DT.bfloat16)
        wq = persist.tile([128, NPIX], DT.bfloat16)
        idxt = persist.tile([128, KF * S16], DT.int16)
        wdup = persist.tile([128, KF * 128], DT.bfloat16)
        sel = persist.tile([128, KF * 4 * 128], DT.bfloat16)
        bdc_t = persist.tile([128, 1], DT.float32)

        x3 = lambda: x_sb[:].rearrange("p (r c) -> p r c", c=PITCH)

        # ======== phase 1: load x (f32 -> bf16), half-split, zero-padded
        nc.vector.memset(x_sb[:], 0.0)
        nc.vector.memset(wq[:], 0.0)
        xv = x_ext[:]
        for h in range(2):
            r0 = max(0, h * HALF - 6)
            r1 = min(H - 1, h * HALF + 69)
            nrow = r1 - r0 + 1
            rloc = r0 - (h * HALF - 6)
            dst = x3()[h * 64 : h * 64 + 64, rloc : rloc + nrow, 1 : 1 + W]
            nc.gpsimd.dma_start(out=dst, in_=xv[:, r0 : r1 + 1, :])

        # ======== phase 2: offset/mask conv
        with (
            tc.tile_pool(name="convw", bufs=1) as convw,
            tc.tile_pool(name="omp", bufs=1) as omp,
            tc.tile_pool(name="convp", bufs=2, space="PSUM") as convp,
        ):
            # w_om views: y/x roles from rows 0..17 (o=2k+r), m role rows 18..26
            wom_yx = wom_ext[:][0:18].rearrange(
                "(o2 r) c kh kw -> c o2 r (kh kw)", r=2
            )
            wom_m = wom_ext[:][18:27].rearrange("o c kh kw -> c o (kh kw)")
            lhs_om = []
            for dy in range(3):
                for dx in range(3):
                    dd = dy * K + dx
                    t = convw.tile([128, 54], DT.bfloat16, tag=f"lom{dd}")
                    nc.vector.memset(t[:], 0.0)
                    for h in range(2):
                        ps = slice(h * 64, h * 64 + 64)
                        nc.gpsimd.dma_start(
                            out=t[ps, h * 27 + 0 : h * 27 + 9],
                            in_=wom_yx[:, 0:9, 0:1, dd : dd + 1].rearrange(
                                "c a b d -> c (a b d)"),
                        )
                        nc.gpsimd.dma_start(
                            out=t[ps, h * 27 + 9 : h * 27 + 18],
                            in_=wom_yx[:, 0:9, 1:2, dd : dd + 1].rearrange(
                                "c a b d -> c (a b d)"),
                        )
                        nc.gpsimd.dma_start(
                            out=t[ps, h * 27 + 18 : h * 27 + 27],
                            in_=wom_m[:, :, dd : dd + 1].rearrange(
                                "c a d -> c (a d)"),
                        )
                    lhs_om.append(t)

            bom_t = convw.tile([54, 1], DT.float32)
            bom_yx = bom_ext[:][0:18].rearrange("(o2 r) -> o2 r", r=2)
            for h in range(2):
                nc.sync.dma_start(
                    out=bom_t[h * 27 + 0 : h * 27 + 9, 0:1], in_=bom_yx[0:9, 0:1]
                )
                nc.sync.dma_start(
                    out=bom_t[h * 27 + 9 : h * 27 + 18, 0:1], in_=bom_yx[0:9, 1:2]
                )
                nc.sync.dma_start(
                    out=bom_t[h * 27 + 18 : h * 27 + 27, 0:1],
                    in_=bom_ext[:][18:27].rearrange("(o one) -> o one", one=1),
                )

            om = omp.tile([54, NPIX], DT.float32)
            rows_per_sub = SUB // W  # 4
            for cb in range(NPIX // SUB):
                pt = convp.tile([54, SUB], DT.float32, tag="cpt")
                r0 = cb * rows_per_sub
                for i, (dy, dx) in enumerate(
                    (dy, dx) for dy in range(3) for dx in range(3)
                ):
                    rhs = x3()[:, 6 + r0 + dy - 1 : 6 + r0 + dy - 1 + rows_per_sub,
                               dx : dx + W]
                    nc.tensor.matmul(
                        out=pt[:], lhsT=lhs_om[i][:], rhs=rhs,
                        start=(i == 0), stop=(i == 8),
                    )
                nc.vector.tensor_scalar(
                    out=om[:, cb * SUB : (cb + 1) * SUB], in0=pt[:],
                    scalar1=bom_t[:, 0:1], scalar2=None, op0=ALU.add,
                )

            # ======== phase 3: coordinate math, chunked, all tiles base-0
            with tc.tile_pool(name="math", bufs=1) as mpool:
                idx16 = mpool.tile([18, NPIX], DT.int16)
                OY = mpool.tile([18, 2048], DT.float32)
                OX = mpool.tile([18, 2048], DT.float32)
                OM = mpool.tile([18, 2048], DT.float32)
                IOT = mpool.tile([18, 2048], DT.float32)
                T0 = mpool.tile([18, 2048], DT.float32)
                T1 = mpool.tile([18, 2048], DT.float32)
                T2 = mpool.tile([18, 2048], DT.float32)
                T3 = mpool.tile([18, 2048], DT.float32)
                cst = mpool.tile([18, 4], DT.float32)

                pidx = mpool.tile([32, 4], DT.float32)
                _iotas.append(nc.gpsimd.iota(pidx[:, 0:1], pattern=[[0, 1]],
                               channel_multiplier=1,
                               allow_small_or_imprecise_dtypes=True))
                P18 = pidx[0:18, 0:1]
                hh, kk, kh3, km3 = (cst[:, i : i + 1] for i in range(4))
                nc.vector.tensor_scalar(out=hh, in0=P18, scalar1=8.5, scalar2=None, op0=ALU.is_gt)
                nc.vector.tensor_scalar(out=kk, in0=hh, scalar1=-9.0, scalar2=None, op0=ALU.mult)
                nc.vector.tensor_add(kk, kk, P18)
                t_a = pidx[0:18, 1:2]
                nc.vector.tensor_scalar(out=kh3, in0=kk, scalar1=2.5, scalar2=None, op0=ALU.is_gt)
                nc.vector.tensor_scalar(out=t_a, in0=kk, scalar1=5.5, scalar2=None, op0=ALU.is_gt)
                nc.vector.tensor_add(kh3, kh3, t_a)
                nc.vector.tensor_scalar(out=km3, in0=kh3, scalar1=-3.0, scalar2=None, op0=ALU.mult)
                nc.vector.tensor_add(km3, km3, kk)
                cstv = mpool.tile([18, 4], DT.float32, tag="cstv")
                nc.vector.tensor_scalar(out=cstv[:, 0:1], in0=hh, scalar1=64.0, scalar2=511.0,
                                        op0=ALU.mult, op1=ALU.add)
                nc.vector.tensor_add(cstv[:, 0:1], cstv[:, 0:1], kh3)
                nc.vector.tensor_scalar(out=cstv[:, 1:2], in0=km3, scalar1=511.0, scalar2=None, op0=ALU.add)
                nc.vector.tensor_scalar(out=cstv[:, 2:3], in0=hh, scalar1=-64.0, scalar2=6.0 - 512.0,
                                        op0=ALU.mult, op1=ALU.add)

                MC = 2048
                for cc in range(NPIX // MC):
                    cs = slice(cc * MC, (cc + 1) * MC)
                    for role, dstt in ((0, OY), (1, OX), (2, OM)):
                        for h in range(2):
                            nc.sync.dma_start(
                                out=dstt[h * 9 : h * 9 + 9, :],
                                in_=om[h * 27 + role * 9 : h * 27 + role * 9 + 9, cs],
                            )
                    # py = OY + rowbase ; fy = mod(py,1); y0f = py - fy
                    _iotas.append(nc.gpsimd.iota(IOT[:], pattern=[[1, MC // W], [0, W]],
                                   base=cc * (MC // W),
                                   channel_multiplier=0,
                                   allow_small_or_imprecise_dtypes=True))
                    nc.vector.tensor_add(T0[:], OY[:], IOT[:])
                    nc.vector.tensor_scalar(out=T0[:], in0=T0[:], scalar1=cstv[:, 0:1],
                                            scalar2=None, op0=ALU.add)
                    nc.vector.tensor_scalar(out=T2[:], in0=T0[:], scalar1=8388608.0, scalar2=-8388608.0,
                                            op0=ALU.add, op1=ALU.add)
                    nc.vector.tensor_tensor(out=OY[:], in0=T2[:], in1=T0[:], op=ALU.is_gt)
                    nc.vector.tensor_sub(T2[:], T2[:], OY[:])
                    nc.vector.tensor_sub(OY[:], T0[:], T2[:])
                    nc.vector.tensor_copy(out=T0[:], in_=T2[:])
                    _iotas.append(nc.gpsimd.iota(IOT[:], pattern=[[0, MC // W], [1, W]],
                                   channel_multiplier=0,
                                   allow_small_or_imprecise_dtypes=True))
                    nc.vector.tensor_add(T1[:], OX[:], IOT[:])
                    nc.vector.tensor_scalar(out=T1[:], in0=T1[:], scalar1=cstv[:, 1:2],
                                            scalar2=None, op0=ALU.add)
                    nc.vector.tensor_scalar(out=T2[:], in0=T1[:], scalar1=8388608.0, scalar2=-8388608.0,
                                            op0=ALU.add, op1=ALU.add)
                    nc.vector.tensor_tensor(out=OX[:], in0=T2[:], in1=T1[:], op=ALU.is_gt)
                    nc.vector.tensor_sub(T2[:], T2[:], OX[:])
                    nc.vector.tensor_sub(OX[:], T1[:], T2[:])
                    nc.vector.tensor_copy(out=T1[:], in_=T2[:])

                    nc.vector.tensor_scalar(out=T0[:], in0=T0[:], scalar1=cstv[:, 2:3],
                                            scalar2=None, op0=ALU.add)
                    nc.vector.tensor_scalar(out=T0[:], in0=T0[:], scalar1=0.0, scalar2=75.0,
                                            op0=ALU.max, op1=ALU.min)
                    nc.vector.tensor_scalar_mul(out=T0[:], in0=T0[:], scalar1=0.5)
                    nc.vector.tensor_scalar(out=T3[:], in0=T0[:], scalar1=8388608.0, scalar2=-8388608.0,
                                            op0=ALU.add, op1=ALU.add)
                    nc.vector.tensor_tensor(out=T2[:], in0=T3[:], in1=T0[:], op=ALU.is_gt)
                    nc.vector.tensor_sub(T3[:], T3[:], T2[:])
                    nc.vector.tensor_sub(T2[:], T0[:], T3[:])
                    nc.vector.tensor_copy(out=T0[:], in_=T3[:])
                    nc.vector.tensor_scalar(out=T1[:], in0=T1[:], scalar1=2.0 - 512.0,
                                            scalar2=None, op0=ALU.add)
                    nc.vector.tensor_scalar(out=T1[:], in0=T1[:], scalar1=0.0, scalar2=130.0,
                                            op0=ALU.max, op1=ALU.min)
                    nc.vector.tensor_scalar_mul(out=T1[:], in0=T1[:], scalar1=0.5)
                    nc.vector.tensor_scalar(out=IOT[:], in0=T1[:], scalar1=8388608.0, scalar2=-8388608.0,
                                            op0=ALU.add, op1=ALU.add)
                    nc.vector.tensor_tensor(out=T3[:], in0=IOT[:], in1=T1[:], op=ALU.is_gt)
                    nc.vector.tensor_sub(IOT[:], IOT[:], T3[:])
                    nc.vector.tensor_sub(T3[:], T1[:], IOT[:])
                    nc.vector.tensor_copy(out=T1[:], in_=IOT[:])

                    nc.vector.tensor_scalar_mul(out=T2[:], in0=T2[:], scalar1=float(4 * RY_N * RX_N))
                    nc.vector.tensor_scalar_mul(out=T3[:], in0=T3[:], scalar1=float(2 * RY_N * RX_N))
                    nc.vector.tensor_add(T2[:], T2[:], T3[:])
                    nc.vector.tensor_scalar_mul(out=T0[:], in0=T0[:], scalar1=float(RX_N))
                    nc.vector.tensor_add(T2[:], T2[:], T0[:])
                    nc.vector.tensor_add(T2[:], T2[:], T1[:])
                    nc.vector.tensor_copy(out=idx16[:, cs], in_=T2[:])

                    nc.scalar.activation(out=OM[:], in_=OM[:], func=AF.Sigmoid)
                    nc.vector.tensor_scalar_mul(out=OM[:], in0=OM[:], scalar1=2.0)
                    nc.vector.tensor_scalar(out=T0[:], in0=OY[:], scalar1=-1.0, scalar2=1.0,
                                            op0=ALU.mult, op1=ALU.add)
                    nc.vector.tensor_scalar(out=T1[:], in0=OX[:], scalar1=-1.0, scalar2=1.0,
                                            op0=ALU.mult, op1=ALU.add)
                    for qi, (ya, xa) in enumerate(((T0, T1), (T0, OX), (OY, T1), (OY, OX))):
                        nc.vector.tensor_mul(T2[:], ya[:], xa[:])
                        nc.vector.tensor_mul(T2[:], T2[:], OM[:])
                        nc.vector.tensor_copy(out=wq[32 * qi : 32 * qi + 18, cs], in_=T2[:])

                # idx16 -> wrapped per-(k,h) layout via DRAM bounce
                idx_dram = nc.dram_tensor("idx_scratch", [18, NPIX], DT.int16)
                nc.sync.dma_start(out=idx_dram[:], in_=idx16[:])
                for k in range(KF):
                    for h in range(2):
                        srcv = idx_dram[h * 9 + k : h * 9 + k + 1, :].rearrange(
                            "p (s l) -> (p l) s", l=16)
                        for g in range(4):
                            p0 = h * 64 + g * 16
                            nc.sync.dma_start(
                                out=idxt[p0 : p0 + 16, k * S16 : (k + 1) * S16],
                                in_=srcv,
                            )

        # selector lhsT: sel[32*qi + j, qi*128 + (j//9)*64 + o] = 1 for j<18
        with tc.tile_pool(name="selb", bufs=1) as selb:
            rP = selb.tile([128, 1], DT.float32)
            cC = selb.tile([128, 512], DT.float32)
            t1 = selb.tile([128, 512], DT.float32)
            t2 = selb.tile([128, 512], DT.float32)
            _iotas.append(nc.gpsimd.iota(rP[:], pattern=[[0, 1]], channel_multiplier=1,
                           allow_small_or_imprecise_dtypes=True))
            _iotas.append(nc.gpsimd.iota(cC[:], pattern=[[1, 512]], channel_multiplier=0,
                           allow_small_or_imprecise_dtypes=True))
            # j = r mod 32 ; qi_r = (r - j)/32
            j32 = selb.tile([128, 1], DT.float32)
            qir = selb.tile([128, 1], DT.float32)
            jt = selb.tile([128, 1], DT.float32)
            nc.vector.tensor_scalar(out=qir[:], in0=rP[:], scalar1=31.5, scalar2=None, op0=ALU.is_gt)
            nc.vector.tensor_scalar(out=jt[:], in0=rP[:], scalar1=63.5, scalar2=None, op0=ALU.is_gt)
            nc.vector.tensor_add(qir[:], qir[:], jt[:])
            nc.vector.tensor_scalar(out=jt[:], in0=rP[:], scalar1=95.5, scalar2=None, op0=ALU.is_gt)
            nc.vector.tensor_add(qir[:], qir[:], jt[:])
            nc.vector.tensor_scalar(out=j32[:], in0=qir[:], scalar1=-32.0, scalar2=None, op0=ALU.mult)
            nc.vector.tensor_add(j32[:], j32[:], rP[:])
            # cond1: floor(c/128) == qi_r  -> |c/128 - qi_r - frac| via mod
            t3 = selb.tile([128, 512], DT.float32)
            nc.vector.tensor_scalar(out=t2[:], in0=cC[:], scalar1=127.5, scalar2=None, op0=ALU.is_gt)
            nc.vector.tensor_scalar(out=t3[:], in0=cC[:], scalar1=255.5, scalar2=None, op0=ALU.is_gt)
            nc.vector.tensor_add(t2[:], t2[:], t3[:])
            nc.vector.tensor_scalar(out=t3[:], in0=cC[:], scalar1=383.5, scalar2=None, op0=ALU.is_gt)
            nc.vector.tensor_add(t2[:], t2[:], t3[:])   # floor(c/128)
            nc.vector.tensor_scalar(out=t1[:], in0=t2[:], scalar1=-128.0, scalar2=None, op0=ALU.mult)
            nc.vector.tensor_add(t1[:], t1[:], cC[:])   # c mod 128
            nc.vector.tensor_scalar(out=t2[:], in0=t2[:], scalar1=qir[:], scalar2=None,
                                    op0=ALU.is_equal)
            # cond2: floor((c mod 128)/64) == floor(j/9)  (j<18 -> floor(j/9) in {0,1})
            nc.vector.tensor_scalar(out=t1[:], in0=t1[:], scalar1=63.5, scalar2=None,
                                    op0=ALU.is_gt)             # h(c)
            hj = selb.tile([128, 1], DT.float32)
            nc.vector.tensor_scalar(out=hj[:], in0=j32[:], scalar1=8.5, scalar2=None,
                                    op0=ALU.is_gt)             # j>=9
            nc.vector.tensor_scalar(out=t1[:], in0=t1[:], scalar1=hj[:], scalar2=None,
                                    op0=ALU.is_equal)
            nc.vector.tensor_mul(t2[:], t2[:], t1[:])
            # cond3: j < 18
            j18 = selb.tile([128, 1], DT.float32)
            nc.vector.tensor_scalar(out=j18[:], in0=j32[:], scalar1=17.5, scalar2=None,
                                    op0=ALU.is_lt)
            nc.vector.tensor_scalar(out=t2[:], in0=t2[:], scalar1=j18[:], scalar2=None,
                                    op0=ALU.mult)
            # per-k selectivity: jk = j32 - 9*hj ; sel_k = t2 * (jk == k)
            jkk = selb.tile([128, 1], DT.float32)
            nc.vector.tensor_scalar(out=jkk[:], in0=hj[:], scalar1=-9.0, scalar2=None,
                                    op0=ALU.mult)
            nc.vector.tensor_add(jkk[:], jkk[:], j32[:])
            tk = selb.tile([128, 1], DT.float32)
            for k in range(KF):
                nc.vector.tensor_scalar(out=tk[:], in0=jkk[:], scalar1=float(k),
                                        scalar2=None, op0=ALU.is_equal)
                nc.vector.tensor_scalar(out=sel[:, k * 512 : (k + 1) * 512],
                                        in0=t2[:], scalar1=tk[:, 0:1],
                                        scalar2=None, op0=ALU.mult)

        # wdup + b_dc
        nc.vector.memset(wdup[:], 0.0)
        wdc_v = wdc_ext[:].rearrange("o c kh kw -> c o (kh kw)")
        for k in range(KF):
            for h in range(2):
                nc.gpsimd.dma_start(
                    out=wdup[h * 64 : h * 64 + 64,
                             k * 128 + h * 64 : k * 128 + h * 64 + 64],
                    in_=wdc_v[:, :, k : k + 1].rearrange("c a d -> c (a d)"),
                )
        for h in range(2):
            nc.sync.dma_start(
                out=bdc_t[h * 64 : h * 64 + 64, 0:1],
                in_=bdc_ext[:].rearrange("(o one) -> o one", one=1),
            )

        # ======== phase 4+5 in one pool scope
        with (
            tc.tile_pool(name="qt", bufs=1) as qtp,
            tc.tile_pool(name="g", bufs=2) as gpool,
            tc.tile_pool(name="h", bufs=2) as hpool,
            tc.tile_pool(name="o", bufs=2) as opool,
            tc.tile_pool(name="mp", bufs=4, space="PSUM") as mpsum,
            tc.tile_pool(name="op", bufs=1, space="PSUM") as opsum,
        ):
            _lib = nc.gpsimd.load_library(library_config.ap_gather)
            for _io in _iotas:
                tile.add_dep_helper(_lib.ins, _io.ins, reason="lib load after iotas")
            qtab = qtp.tile([128, NBLK * 4], DT.bfloat16)
            nc.vector.memset(qtab[:], 0.0)
            q4 = qtab[:].rearrange("p (blk q) -> p blk q", q=4)
            for a in range(2):
                for b in range(2):
                    blk0 = (a * 2 + b) * (RY_N * RX_N)
                    for qy in range(2):
                        for qx in range(2):
                            ry_cnt = min((75 - a - qy) // 2 + 1, RY_N)
                            rx0 = 1 if (b + qx) == 0 else 0
                            rx1 = min(RX_N - 1, (130 - b - qx) // 2)
                            rx_cnt = rx1 - rx0 + 1
                            c0 = 2 * rx0 + b + qx - 1
                            src = x3()[:, a + qy : a + qy + 2 * (ry_cnt - 1) + 1 : 2,
                                       c0 : c0 + 2 * (rx_cnt - 1) + 1 : 2]
                            dst3 = q4[:, blk0 + rx0 : blk0 + rx0 + (ry_cnt - 1) * RX_N + rx_cnt,
                                      qy * 2 + qx : qy * 2 + qx + 1]
                            dst = bass.AP(
                                dst3.tensor, dst3.offset,
                                [dst3.ap[0], [RX_N * 4, ry_cnt], [4, rx_cnt]],
                            )
                            nc.vector.tensor_copy(out=dst, in_=src)

            qtab_f32 = qtab[:].bitcast(DT.float32)
            outv = out_ext[:].rearrange("o h w -> o (h w)")

            for cb in range(NPIX // GCHUNK):
                po = opsum.tile([128, GCHUNK], DT.float32, tag="po")
                for k in range(KF):
                    g = gpool.tile([128, GCHUNK * 2], DT.float32, tag="g")
                    idx_sl = idxt[:, k * S16 + cb * (GCHUNK // 16):
                                  k * S16 + (cb + 1) * (GCHUNK // 16)]
                    _ga = nc.gpsimd.ap_gather(
                        g[:], qtab_f32, idx_sl,
                        channels=128, num_elems=NBLK, d=2, num_idxs=GCHUNK,
                    )
                    tile.add_dep_helper(_ga.ins, _lib.ins, reason="gather after lib load")
                    gb = g[:].bitcast(DT.bfloat16).rearrange(
                        "p (n q) -> p n q", q=4
                    )
                    for sub in range(GCHUNK // SUB):
                        col0 = cb * GCHUNK + sub * SUB
                        hts = []
                        for qi in range(4):
                            mq = mpsum.tile([128, SUB], DT.float32, tag="mq")
                            nc.tensor.matmul(
                                out=mq[:],
                                lhsT=sel[:, k * 512 + qi * 128 : k * 512 + (qi + 1) * 128],
                                rhs=wq[:, col0 : col0 + SUB],
                                start=True, stop=True,
                            )
                            ht = hpool.tile([128, SUB], DT.bfloat16, tag=f"ht{qi}")
                            gq = gb[:, sub * SUB : (sub + 1) * SUB,
                                    qi : qi + 1].rearrange("p n one -> p (n one)")
                            nc.vector.tensor_mul(ht[:], mq[:], gq)
                            hts.append(ht)
                        for qi in range(4):
                            nc.tensor.matmul(
                                out=po[:, sub * SUB : (sub + 1) * SUB],
                                lhsT=wdup[:, k * 128 : (k + 1) * 128],
                                rhs=hts[qi][:],
                                start=(k == 0 and qi == 0),
                                stop=(k == KF - 1 and qi == 3),
                            )
                ot = opool.tile([128, GCHUNK], DT.float32, tag="ot")
                nc.vector.tensor_scalar(
                    out=ot[:], in0=po[:], scalar1=bdc_t[:, 0:1],
                    scalar2=None, op0=ALU.add,
                )
                for h in range(2):
                    nc.sync.dma_start(
                        out=outv[:, h * NPIX + cb * GCHUNK:
                                 h * NPIX + (cb + 1) * GCHUNK],
                        in_=ot[h * 64 : h * 64 + 64, :],
                    )


def _build_nc():
    _install_compat()
    nc = bass.Bass()
    x_ext = nc.declare_dram_parameter("x", [C, H, W], DT.float32, isOutput=False)
    wom_ext = nc.declare_dram_parameter("w_om", [3 * KF, C, K, K], DT.float32, isOutput=False)
    bom_ext = nc.declare_dram_parameter("b_om", [3 * KF], DT.float32, isOutput=False)
    wdc_ext = nc.declare_dram_parameter("w_dc", [O, C, K, K], DT.float32, isOutput=False)
    bdc_ext = nc.declare_dram_parameter("b_dc", [O], DT.float32, isOutput=False)
    out_ext = nc.declare_dram_parameter("out", [O, H, W], DT.float32, isOutput=True)
    with tile.TileContext(nc) as tc:
        _emit(nc, tc, x_ext, wom_ext, bom_ext, wdc_ext, bdc_ext, out_ext)
    lower_extended_insts(nc)
    return nc


_NC_CACHE = None


def kernel(**inputs):
    global _NC_CACHE
    x = np.ascontiguousarray(inputs["x"], dtype=np.float32)
    w_om = np.ascontiguousarray(inputs["w_om"], dtype=np.float32)
    b_om = np.ascontiguousarray(inputs["b_om"], dtype=np.float32)
    w_dc = np.ascontiguousarray(inputs["w_dc"], dtype=np.float32)
    b_dc = np.ascontiguousarray(inputs["b_dc"], dtype=np.float32)

    if _NC_CACHE is None:
        _NC_CACHE = _build_nc()
    nc = _NC_CACHE

    in_maps = [
        {"x": x[i], "w_om": w_om, "b_om": b_om, "w_dc": w_dc, "b_dc": b_dc}
        for i in range(NCORES)
    ]
    res = run_bass_kernel_spmd(nc, in_maps, core_ids=list(range(NCORES)))
    return np.stack(
        [np.asarray(res.results[i]["out"]) for i in range(NCORES)]
    ).astype(np.float32)



# revision 8
# speedup vs baseline: 1.4404x; 1.4404x over previous
"""Trainium2 Bass kernel for nn_AdaFeatBlock (modulated deformable-conv block).

Sharding: data-parallel over batch - 8 samples -> 8 NeuronCores, all weights
replicated (host-prepacked into device-friendly layouts); each core computes
its sample end-to-end, host stacks outputs.

Per-core pipeline (one sample, x [64,128,128]):
  1. x -> bf16 "half-split" padded layout x_sb: partition h*64+c; free =
     76 stored rows (half rows -6..69) x 130 cols (-1..128), zero borders.
  2. offset/mask 3x3 conv: 9 shifted matmuls per 512-px block with a
     host-packed block-diagonal lhsT [128, 54] (row order role*18+h*9+k),
     PSUM-accumulated; each block's PSUM is DMA-scattered into math-layout
     tiles OY/OX/OM [72, 2048] (partition = chunk*18 + h*9 + k).
  3. Coordinate math on [72, 2048] tiles (all 4 pixel-chunks at once in the
     partition dim): bilinear corner weights -> W4 [72, 4qi*2048] bf16 and
     quad-table indices -> IDX [72, 2048] i16.
  4. IDX -> DRAM bounce -> idxt [128, 36*128] i16 in ap_gather stream
     layout: per (cb,k) call, partition j of each 16-partition group holds
     the indices of pixels cb*2048 + j*128 .. +127 (stream u = s*16+j).
  5. Quad gather table Q[128, 10032*4] bf16 (2x2 pixel blocks at 4 row/col
     parities, built by Act-engine strided copies); ap_gather (d=2 f32 view
     = 8B quad) fetches 2048 px * 4 corners for all 128 partitions.
  6. Per (cb,k,sub): selector matmul broadcasts W4 rows quad-minor into
     PSUM [128, 2048]; Act copies PSUM->bf16 (some subs); DVE multiplies
     with gathered quads; 4 matmuls with block-diag channel-duplicated w_dc
     accumulate over (k,qi) into po PSUM.
  7. Act adds b_dc and un-permutes stream->pixel order; DMA out.
"""

import numpy as np
import ml_dtypes

import concourse.bass as bass
import concourse.tile as tile
from concourse import mybir
from concourse.bass_utils import run_bass_kernel_spmd
from concourse import library_config
from concourse.library_overlay import lower_extended_insts
from concourse.vector_clock import ScopedClock

AF = mybir.ActivationFunctionType
ALU = mybir.AluOpType
DT = mybir.dt

B, C, H, W = 8, 64, 128, 128
O = 64
K = 3
KF = 9
NCORES = 8
HALF = H // 2
NPIX = H * W // 2              # 8192 pixels per half
ROWS_ST = 76                   # stored rows per half
PITCH = 130                    # stored cols (-1..128)
RY_N, RX_N = 38, 66
RR = RY_N * RX_N               # 2508
NBLK = 4 * RR                  # 10032
NCH = 4                        # pixel chunks per half
MC = NPIX // NCH               # 2048 px per chunk
SUB = 512
MROW = 2 * KF * NCH            # 72 math rows
MAGIC = 8388608.0              # 2^23 round-to-int magic

# which subs (of 4 per (cb,k)) go through the Act-engine PSUM->bf16 copy
ACT_SUBS = (0, 1, 2, 3)

BF16 = ml_dtypes.bfloat16


def _install_compat():
    """This walrus build accepts at most ONE sync-wait per instruction."""
    if getattr(tile.TileContext, "_adafeat_patched", False):
        return
    _orig_lower = tile.TileContext._lower_ordered_insts

    def _split_waits(nc, ordered):
        for insts in ordered.values():
            new_insts = []
            for inst in insts:
                si = inst.sync_info
                if si is not None and si.on_wait and len(si.on_wait) > 1:
                    waits = list(si.on_wait)
                    for w in waits[:-1]:
                        nop = mybir.InstNoOp(name=f"I-{nc.next_id()}", ins=[], outs=[])
                        nop.engine = inst.engine
                        nop.sync_info = mybir.SyncInfo(on_wait=[w], on_update=[])
                        new_insts.append(nop)
                    inst.sync_info = mybir.SyncInfo(
                        on_wait=[waits[-1]], on_update=list(si.on_update)
                    )
                new_insts.append(inst)
            insts[:] = new_insts

    def _lower_split(self, ordered):
        _split_waits(self.nc, ordered)
        return _orig_lower(self, ordered)

    def _drain_split(self, tick_clock, wait_clock):
        carrier = self.nc.sync.nop(nofuse=True)
        wait_clock.add_sem_waits(
            carrier.ins, ScopedClock({None: tick_clock.global_clock})
        )
        si = carrier.ins.sync_info
        if si is not None and si.on_wait and len(si.on_wait) > 1:
            waits = list(si.on_wait)
            carrier.ins.sync_info = mybir.SyncInfo(
                on_wait=waits[:1], on_update=list(si.on_update)
            )
            for w in waits[1:]:
                extra = self.nc.sync.nop(nofuse=True)
                extra.ins.sync_info = mybir.SyncInfo(on_wait=[w], on_update=[])
        self.nc.sync.drain()
        self.nc.all_engine_barrier()
        popped = self.nc._tile_sem_poison_stack.pop()
        assert popped is self._sem_poison
        self.nc.clear_and_free_semaphores(list(self.sems.allocated().values()))
        self.nc.all_engine_barrier()

    tile.TileContext._lower_ordered_insts = _lower_split
    tile.TileContext._drain_and_barrier = _drain_split
    tile.TileContext._adafeat_patched = True


def _fap(v, extra_off, dims):
    """AP with custom free dims on an SBUF/PSUM tile view (strides in elems)."""
    return bass.AP(v.tensor, v.offset + extra_off, [v.ap[0]] + dims)


def _emit(nc, tc, ext):
    x_ext = ext["x"]
    out_ext = ext["out"]

    with tc.tile_pool(name="persist", bufs=1) as persist:
        qtab = persist.tile([128, NBLK * 4], DT.bfloat16)
        idxt = persist.tile([128, KF * NCH * 128], DT.int16)
        W4 = persist.tile([MROW, 4 * MC], DT.bfloat16)
        wdup = persist.tile([128, KF * 128], DT.bfloat16)
        sel36 = persist.tile([MROW, KF * NCH * 128], DT.bfloat16)
        lhs_om = persist.tile([128, KF * 54], DT.bfloat16)
        cst = persist.tile([MROW, 4], DT.float32)
        iot2 = persist.tile([MROW, 2 * MC], DT.bfloat16)
        bdc_t = persist.tile([128, 1], DT.float32)

        # param loads (contiguous, few big descriptors each)
        nc.sync.dma_start(out=wdup[:], in_=ext["wdup"][:])
        nc.sync.dma_start(out=sel36[:], in_=ext["sel36"][:])
        nc.sync.dma_start(out=lhs_om[:], in_=ext["lhs_om"][:])
        nc.sync.dma_start(out=cst[:], in_=ext["cst"][:])
        nc.sync.dma_start(out=iot2[:], in_=ext["iot2"][:])
        nc.sync.dma_start(out=bdc_t[:], in_=ext["bdc_t"][:])

        # qtab memset on the (otherwise idle) gpsimd queue
        nc.gpsimd.memset(qtab[:], 0.0)
        q4 = qtab[:].rearrange("p (blk q) -> p blk q", q=4)

        idx_dram = nc.dram_tensor("idx_scratch", [MROW, MC], DT.int16)

        with tc.tile_pool(name="pmain", bufs=1) as pmain:
            OY = pmain.tile([MROW, MC], DT.float32)
            OX = pmain.tile([MROW, MC], DT.float32)
            OM = pmain.tile([MROW, MC], DT.float32)
            OMs = pmain.tile([MROW, MC], DT.bfloat16)
            IDX = pmain.tile([MROW, MC], DT.int16)

            with (
                tc.tile_pool(name="px", bufs=1) as px,
                tc.tile_pool(name="convp", bufs=8, space="PSUM") as convp,
            ):
                x_sb = px.tile([128, ROWS_ST * PITCH], DT.bfloat16)
                x3 = lambda: x_sb[:].rearrange("p (r c) -> p r c", c=PITCH)

                # zero borders only: top/bottom halo rows + left/right cols
                nc.vector.memset(x3()[0:64, 0:6, :], 0.0)
                nc.vector.memset(x3()[64:128, 70:76, :], 0.0)
                nc.vector.memset(x3()[:, :, 0:1], 0.0)
                nc.vector.memset(x3()[:, :, 129:130], 0.0)

                xv = x_ext[:]
                for h in range(2):
                    r0 = max(0, h * HALF - 6)
                    r1 = min(H - 1, h * HALF + 69)
                    nrow = r1 - r0 + 1
                    rloc = r0 - (h * HALF - 6)
                    dst = x3()[h * 64 : h * 64 + 64, rloc : rloc + nrow, 1 : 1 + W]
                    nc.gpsimd.dma_start(out=dst, in_=xv[:, r0 : r1 + 1, :])

                _lib = nc.gpsimd.load_library(library_config.ap_gather)

                # ---- offset/mask conv: 2 passes x 8 blocks, tap-outer ----
                for grp in range(2):
                    pts = [
                        convp.tile([54, SUB], DT.float32, tag="cpt", name=f"cpt{b}")
                        for b in range(8)
                    ]
                    for i in range(KF):
                        dy, dx = i // 3, i % 3
                        for bi in range(8):
                            blk = grp * 8 + bi
                            r0 = blk * 4
                            rhs = x3()[:, 6 + r0 + dy - 1 : 6 + r0 + dy + 3,
                                       dx : dx + W]
                            nc.tensor.matmul(
                                out=pts[bi][:],
                                lhsT=lhs_om[:, i * 54 : (i + 1) * 54],
                                rhs=rhs,
                                start=(i == 0), stop=(i == KF - 1),
                            )
                    for bi in range(8):
                        blk = grp * 8 + bi
                        cb2, po_ = blk // 4, (blk % 4) * SUB
                        ob = px.tile([54, SUB], DT.float32, tag="ob", name="ob",
                                     bufs=4)
                        nc.scalar.activation(out=ob[:], in_=pts[bi][:], func=AF.Copy)
                        for role, dstt in ((0, OY), (1, OX), (2, OM)):
                            nc.sync.dma_start(
                                out=dstt[cb2 * 18 : cb2 * 18 + 18, po_ : po_ + SUB],
                                in_=ob[role * 18 : role * 18 + 18, :],
                            )

                # ---- quad gather table from x_sb (Act engine copies) ----
                for a in range(2):
                    for b in range(2):
                        blk0 = (a * 2 + b) * RR
                        for qy in range(2):
                            for qx in range(2):
                                ry_cnt = min((75 - a - qy) // 2 + 1, RY_N)
                                rx0 = 1 if (b + qx) == 0 else 0
                                rx1 = min(RX_N - 1, (130 - b - qx) // 2)
                                rx_cnt = rx1 - rx0 + 1
                                c0 = 2 * rx0 + b + qx - 1
                                src = x3()[:, a + qy : a + qy + 2 * (ry_cnt - 1) + 1 : 2,
                                           c0 : c0 + 2 * (rx_cnt - 1) + 1 : 2]
                                dst3 = q4[:, blk0 + rx0 : blk0 + rx0
                                          + (ry_cnt - 1) * RX_N + rx_cnt,
                                          qy * 2 + qx : qy * 2 + qx + 1]
                                dst = bass.AP(
                                    dst3.tensor, dst3.offset,
                                    [dst3.ap[0], [RX_N * 4, ry_cnt], [4, rx_cnt]],
                                )
                                nc.scalar.activation(out=dst, in_=src, func=AF.Copy)

            # ---- coordinate math on [72, 2048] ----
            with tc.tile_pool(name="ptmp", bufs=1) as ptmp:
                TA = ptmp.tile([MROW, MC], DT.float32)
                TB = ptmp.tile([MROW, MC], DT.float32)
                TC_ = ptmp.tile([MROW, MC], DT.float32)
                TD = ptmp.tile([MROW, MC], DT.float32)

                ts = nc.vector.tensor_scalar
                tt = nc.vector.tensor_tensor
                stt = nc.vector.scalar_tensor_tensor

                # mask = sigmoid(om_m + b_om_m) on Act (x2 folded into wdup)
                nc.scalar.activation(out=OMs[:], in_=OM[:], func=AF.Sigmoid,
                                     bias=cst[:, 2:3], scale=1.0)

                # y-pass: P = OY + cst_y + iota_row
                stt(out=TA[:], in0=OY[:], scalar=cst[:, 0:1], in1=iot2[:, 0:MC],
                    op0=ALU.add, op1=ALU.add)
                ts(out=TB[:], in0=TA[:], scalar1=MAGIC, scalar2=-MAGIC,
                   op0=ALU.add, op1=ALU.add)
                tt(out=TC_[:], in0=TB[:], in1=TA[:], op=ALU.is_gt)
                tt(out=OY[:], in0=TB[:], in1=TC_[:], op=ALU.subtract)   # y0_local
                tt(out=TB[:], in0=TA[:], in1=OY[:], op=ALU.subtract)    # fy
                ts(out=OY[:], in0=OY[:], scalar1=0.0, scalar2=75.0,
                   op0=ALU.max, op1=ALU.min)
                nc.vector.tensor_scalar_mul(out=TA[:], in0=OY[:], scalar1=0.5)
                ts(out=TC_[:], in0=TA[:], scalar1=MAGIC, scalar2=-MAGIC,
                   op0=ALU.add, op1=ALU.add)
                tt(out=OY[:], in0=TC_[:], in1=TA[:], op=ALU.is_gt)
                tt(out=TC_[:], in0=TC_[:], in1=OY[:], op=ALU.subtract)  # ry
                tt(out=TA[:], in0=TA[:], in1=TC_[:], op=ALU.subtract)   # pa_y/2

                # x-pass: P = OX + cst_x + iota_col  (value = x0_stored+1 dance)
                stt(out=TD[:], in0=OX[:], scalar=cst[:, 1:2], in1=iot2[:, MC : 2 * MC],
                    op0=ALU.add, op1=ALU.add)
                ts(out=OX[:], in0=TD[:], scalar1=MAGIC, scalar2=-MAGIC,
                   op0=ALU.add, op1=ALU.add)
                tt(out=OM[:], in0=OX[:], in1=TD[:], op=ALU.is_gt)
                tt(out=OX[:], in0=OX[:], in1=OM[:], op=ALU.subtract)    # x0_stored+1
                tt(out=TD[:], in0=TD[:], in1=OX[:], op=ALU.subtract)    # fx
                ts(out=OX[:], in0=OX[:], scalar1=0.0, scalar2=130.0,
                   op0=ALU.max, op1=ALU.min)
                nc.vector.tensor_scalar_mul(out=OM[:], in0=OX[:], scalar1=0.5)
                ts(out=OX[:], in0=OM[:], scalar1=MAGIC, scalar2=-MAGIC,
                   op0=ALU.add, op1=ALU.add)
                tt(out=OY[:], in0=OX[:], in1=OM[:], op=ALU.is_gt)
                tt(out=OX[:], in0=OX[:], in1=OY[:], op=ALU.subtract)    # rx
                tt(out=OM[:], in0=OM[:], in1=OX[:], op=ALU.subtract)    # pa_x/2

                # idx = pa_y*4RR + pa_x*2RR + ry*RX_N + rx  (pa_* are half-parities)
                stt(out=OY[:], in0=TC_[:], scalar=float(RX_N), in1=OX[:],
                    op0=ALU.mult, op1=ALU.add)
                stt(out=TC_[:], in0=TA[:], scalar=float(4 * RR), in1=OY[:],
                    op0=ALU.mult, op1=ALU.add)
                stt(out=OY[:], in0=OM[:], scalar=float(2 * RR), in1=TC_[:],
                    op0=ALU.mult, op1=ALU.add)
                nc.vector.tensor_copy(out=IDX[:], in_=OY[:])

                # idx bounce: SBUF -> DRAM -> stream-layout idxt
                nc.sync.dma_start(out=idx_dram[:], in_=IDX[:])
                dv = idx_dram[:]
                for h in range(2):
                    for g in range(4):
                        p0 = h * 64 + g * 16
                        for cb in range(NCH):
                            src = bass.AP(
                                dv.tensor, dv.offset + (cb * 18 + h * KF) * MC,
                                [[128, 16], [MC, KF], [1, 128]],
                            )
                            dst = idxt[p0 : p0 + 16,
                                       cb * KF * 128 : (cb + 1) * KF * 128
                                       ].rearrange("p (k s) -> p k s", k=KF)
                            nc.sync.dma_start(out=dst, in_=src)

                # corner weights -> W4 (quad-minor per qi block)
                nc.scalar.activation(out=TC_[:], in_=TB[:], func=AF.Copy,
                                     bias=1.0, scale=-1.0)               # 1-fy
                nc.scalar.activation(out=OX[:], in_=TD[:], func=AF.Copy,
                                     bias=1.0, scale=-1.0)               # 1-fx
                for qy in range(2):
                    for qx in range(2):
                        qi = qy * 2 + qx
                        ya = TB if qy else TC_
                        xa = TD if qx else OX
                        tt(out=OY[:], in0=ya[:], in1=xa[:], op=ALU.mult)
                        tt(out=W4[:, qi * MC : (qi + 1) * MC], in0=OY[:],
                           in1=OMs[:], op=ALU.mult)

        # ---- phase 4: gather + broadcast + modulate + deform matmul ----
        qtab_f32 = qtab[:].bitcast(DT.float32)
        outv = out_ext[:].rearrange("o h w -> o (h w)")
        W4v = W4[:]

        with (
            tc.tile_pool(name="g", bufs=2) as gpool,
            tc.tile_pool(name="mqs", bufs=2) as mqsp,
            tc.tile_pool(name="ht", bufs=2) as htp,
            tc.tile_pool(name="ot", bufs=2) as otp,
            tc.tile_pool(name="mp", bufs=1, space="PSUM") as mpsum,
            tc.tile_pool(name="op", bufs=1, space="PSUM") as opsum,
        ):
            for cb in range(NCH):
                po = opsum.tile([128, MC], DT.float32, tag="po")
                for k in range(KF):
                    g = gpool.tile([128, MC * 2], DT.float32, tag="g")
                    idx_sl = idxt[:, (cb * KF + k) * 128 : (cb * KF + k + 1) * 128]
                    _ga = nc.gpsimd.ap_gather(
                        g[:], qtab_f32, idx_sl,
                        channels=128, num_elems=NBLK, d=2, num_idxs=MC,
                    )
                    tile.add_dep_helper(_ga.ins, _lib.ins, reason="gather after lib")
                    gb = g[:].bitcast(DT.bfloat16)
                    sel_sl = sel36[:, (cb * KF + k) * 128 : (cb * KF + k + 1) * 128]

                    for sub in range(4):
                        mq4 = mpsum.tile([128, 2048], DT.float32, tag="mq")
                        for mm in range(4):
                            rhs = _fap(W4v, sub * 32 + mm * 8,
                                       [[1, 8], [128, 16], [MC, 4]])
                            nc.tensor.matmul(
                                out=mq4[:, mm * 512 : (mm + 1) * 512],
                                lhsT=sel_sl, rhs=rhs,
                                start=True, stop=True,
                            )
                        ht = htp.tile([128, 2048], DT.bfloat16, tag="ht")
                        gsl = gb[:, sub * 2048 : (sub + 1) * 2048]
                        if sub in ACT_SUBS:
                            mqs = mqsp.tile([128, 2048], DT.bfloat16, tag="mqs")
                            nc.scalar.activation(out=mqs[:], in_=mq4[:], func=AF.Copy)
                            nc.vector.tensor_tensor(out=ht[:], in0=mqs[:], in1=gsl,
                                                    op=ALU.mult)
                        else:
                            nc.vector.tensor_tensor(out=ht[:], in0=mq4[:], in1=gsl,
                                                    op=ALU.mult)
                        htv = ht[:]
                        for qi in range(4):
                            nc.tensor.matmul(
                                out=po[:, sub * SUB : (sub + 1) * SUB],
                                lhsT=wdup[:, k * 128 : (k + 1) * 128],
                                rhs=bass.AP(htv.tensor, htv.offset + qi,
                                            [htv.ap[0], [4, 512]]),
                                start=(k == 0 and qi == 0),
                                stop=(k == KF - 1 and qi == 3),
                            )

                # bias + stream->pixel unpermute (Act), then store
                ot2 = otp.tile([128, MC], DT.float32, tag="ot")
                pov = po[:]
                nc.scalar.activation(
                    out=ot2[:],
                    in_=bass.AP(pov.tensor, pov.offset, [pov.ap[0], [1, 16], [16, 128]]),
                    func=AF.Identity, bias=bdc_t[:, 0:1], scale=1.0,
                )
                for h in range(2):
                    nc.sync.dma_start(
                        out=outv[:, h * NPIX + cb * MC : h * NPIX + (cb + 1) * MC],
                        in_=ot2[h * 64 : h * 64 + 64, :],
                    )


def _pack_params(w_om, b_om, w_dc, b_dc):
    # conv lhsT tiles [128, 9*54]: row h*64+c, col dd*54 + (role*18 + h*9 + k)
    lhs = np.zeros((128, KF * 54), np.float32)
    for dd in range(KF):
        dy, dx = dd // 3, dd % 3
        for h in range(2):
            for kk in range(KF):
                lhs[h * 64 : h * 64 + 64, dd * 54 + 0 * 18 + h * 9 + kk] = \
                    w_om[2 * kk, :, dy, dx]
                lhs[h * 64 : h * 64 + 64, dd * 54 + 1 * 18 + h * 9 + kk] = \
                    w_om[2 * kk + 1, :, dy, dx]
                lhs[h * 64 : h * 64 + 64, dd * 54 + 2 * 18 + h * 9 + kk] = \
                    w_om[18 + kk, :, dy, dx]

    # selector one-hots [72, 36*128]
    sel = np.zeros((MROW, KF * NCH * 128), np.float32)
    for cb in range(NCH):
        for k in range(KF):
            for p in range(128):
                h = p // 64
                sel[cb * 18 + h * 9 + k, (cb * KF + k) * 128 + p] = 1.0

    # wdup [128, 9*128] with the mask's 2.0 folded in
    wd = np.zeros((128, KF * 128), np.float32)
    for k in range(KF):
        kh, kw = k // 3, k % 3
        for h in range(2):
            wd[h * 64 : h * 64 + 64, k * 128 + h * 64 : k * 128 + h * 64 + 64] = \
                2.0 * w_dc[:, :, kh, kw].T

    # per-row constants [72, 4]: cst_y, cst_x, mask bias
    cstv = np.zeros((MROW, 4), np.float32)
    for p in range(MROW):
        cb, hk = divmod(p, 18)
        h, kk = divmod(hk, 9)
        kh, kw = kk // 3, kk % 3
        cstv[p, 0] = b_om[2 * kk] + cb * 16 + kh + 5
        cstv[p, 1] = b_om[2 * kk + 1] + kw + 1
        cstv[p, 2] = b_om[18 + kk]

    # iotas [72, 2*2048]: row (u//128) then col (u%128); exact in bf16
    u = np.arange(MC)
    iot = np.concatenate([u // 128, u % 128]).astype(np.float32)
    iot2 = np.broadcast_to(iot, (MROW, 2 * MC)).copy()

    bdc = np.zeros((128, 1), np.float32)
    bdc[:64, 0] = b_dc
    bdc[64:, 0] = b_dc

    return {
        "lhs_om": lhs.astype(BF16),
        "sel36": sel.astype(BF16),
        "wdup": wd.astype(BF16),
        "cst": cstv,
        "iot2": iot2.astype(BF16),
        "bdc_t": bdc,
    }


def _build_nc():
    _install_compat()
    nc = bass.Bass()
    ext = {}
    ext["x"] = nc.declare_dram_parameter("x", [C, H, W], DT.float32, isOutput=False)
    ext["lhs_om"] = nc.declare_dram_parameter("lhs_om", [128, KF * 54], DT.bfloat16, isOutput=False)
    ext["sel36"] = nc.declare_dram_parameter("sel36", [MROW, KF * NCH * 128], DT.bfloat16, isOutput=False)
    ext["wdup"] = nc.declare_dram_parameter("wdup", [128, KF * 128], DT.bfloat16, isOutput=False)
    ext["cst"] = nc.declare_dram_parameter("cst", [MROW, 4], DT.float32, isOutput=False)
    ext["iot2"] = nc.declare_dram_parameter("iot2", [MROW, 2 * MC], DT.bfloat16, isOutput=False)
    ext["bdc_t"] = nc.declare_dram_parameter("bdc_t", [128, 1], DT.float32, isOutput=False)
    ext["out"] = nc.declare_dram_parameter("out", [O, H, W], DT.float32, isOutput=True)
    with tile.TileContext(nc) as tc:
        _emit(nc, tc, ext)
    lower_extended_insts(nc)
    return nc


_NC_CACHE = None


def kernel(**inputs):
    global _NC_CACHE
    x = np.ascontiguousarray(inputs["x"], dtype=np.float32)
    w_om = np.ascontiguousarray(inputs["w_om"], dtype=np.float32)
    b_om = np.ascontiguousarray(inputs["b_om"], dtype=np.float32)
    w_dc = np.ascontiguousarray(inputs["w_dc"], dtype=np.float32)
    b_dc = np.ascontiguousarray(inputs["b_dc"], dtype=np.float32)

    if _NC_CACHE is None:
        _NC_CACHE = _build_nc()
    nc = _NC_CACHE

    packed = _pack_params(w_om, b_om, w_dc, b_dc)
    in_maps = [{"x": x[i], **packed} for i in range(NCORES)]
    res = run_bass_kernel_spmd(nc, in_maps, core_ids=list(range(NCORES)))
    return np.stack(
        [np.asarray(res.results[i]["out"]) for i in range(NCORES)]
    ).astype(np.float32)
